# revision 26
# baseline (speedup 1.0000x reference)
"""Distributed Bass kernel for GQA causal attention (B=2, S=2048, H=2048,
NH=16, NKV=4, HD=128) on 8 TRN2 NeuronCores.

Sharding: core c (0..7) handles batch b = c//4 and kv-group g = c%4
(4 query heads + 1 kv head, GQA groups kept intact).  wq/wk/wv are
column-sharded, wo row-sharded; each core emits a partial output
[H, S] (transposed, bf16) and the host sums the 4 group-partials per
batch in f32.

v2 design notes (vs the f32r v1):
  - All matmuls in bf16: full-rate streaming at ANY moving width
    (f32r drops to 1/4 rate under 256 cols -- the causal-diagonal
    tiles), half DMA traffic / SBUF footprint / LDWEIGHTS size.
  - Matmul ISA limits respected: every matmul out is a 2D <=512-col
    single-PSUM-bank region; head-pairs share a [P, 2, SB] PSUM tile
    so ACT/DVE/Pool touch both heads with one (3D-AP) instruction.
  - Elementwise work spread across engines: ACT (exp, PSUM->SBUF
    staging), DVE (rot*sin, adds, 128-lane reciprocals, U staging),
    Pool==nc.gpsimd (raw*cos, causal mask multiply, final normalize).
  - V transposed to [s, d] 128-blocks by DMA-transpose (XBAR), not PE.
  - Attention runs kj-outer with a one-step software pipeline: the
    score matmul of tile kj+1 issues before the PV/rowsum of kj, so
    the ACT exp latency never stalls the PE.  Causal mask is a
    multiplicative 0/1 bf16 on P after exp (Pool engine).
  - Per-block softmax normalization (U copy, rowsum staging, PE
    broadcast matmuls, reciprocal, U*(1/r)) is deferred into the next
    block's kj loop -- fully off the PE critical path.
  - Output projection accumulates into [P, 2, SB] tiles (2 s-blocks),
    staged to SBUF alternating ACT/DVE, DMA'd out as bf16.
"""

import math
import os
import sys

import ml_dtypes
import numpy as np

sys.path.insert(0, "/opt/trn_rl_repo")

import concourse.bass as bass
import concourse.mybir as mybir
import concourse.tile as tile
from concourse.bass_utils import run_bass_kernel_spmd

B, S, H = 2, 2048, 2048
NH, NKV, HD = 16, 4, 128
NCORES = 8
GH = 4                # q-heads per core (one kv group)
P = 128
SB = 512              # s-block width (single PSUM bank of f32)
NB = S // SB          # 4 s-blocks
NT = S // P           # 16 partition tiles along s / h / e
SCALE = 1.0 / math.sqrt(HD)
F32 = mybir.dt.float32
BF16 = mybir.dt.bfloat16
NPDT = ml_dtypes.bfloat16


def _consts():
    # rotate_half as matmul: rot = RT.T @ q  (RT is the lhsT)
    RT = np.zeros((P, P), NPDT)
    idx = np.arange(64)
    RT[idx + 64, idx] = -1.0
    RT[idx, idx + 64] = 1.0
    # multiplicative causal mask for the diagonal P x P block:
    # P[kj, qi] valid iff kj <= qi
    kjl = np.arange(P)[:, None]
    qil = np.arange(P)[None, :]
    mask01 = (kjl <= qil).astype(NPDT)
    ones_k = np.ones((P, 1), NPDT)
    ones_1 = np.ones((1, P), NPDT)
    return RT, mask01, ones_k, ones_1


def build_nc():
    nc = bass.Bass()

    xT_d = nc.declare_dram_parameter("xT", [H, S], BF16, isOutput=False)
    # host packs [wq | wk | wv] -> one [H, 768] param (fewer DMA issues)
    wqkv_d = nc.declare_dram_parameter("wqkv", [H, (GH + 2) * HD], BF16,
                                       isOutput=False)
    wo_d = nc.declare_dram_parameter("wo", [GH * HD, H], BF16, isOutput=False)
    cosT_d = nc.declare_dram_parameter("cosT", [HD, S], BF16, isOutput=False)
    sinT_d = nc.declare_dram_parameter("sinT", [HD, S], BF16, isOutput=False)
    out_d = nc.declare_dram_parameter("out", [H, S], BF16, isOutput=True)

    RT_np, mask01_np, ones_k_np, ones_1_np = _consts()
    RT_d = nc.inline_tensor(RT_np, "rot_t")
    mask01_d = nc.inline_tensor(mask01_np, "mask01")
    ones_k_d = nc.inline_tensor(ones_k_np, "ones_k")
    ones_1_d = nc.inline_tensor(ones_1_np, "ones_1")

    with tile.TileContext(nc) as tc, \
         tc.tile_pool(name="persist", bufs=1) as persist:
        rt_sb = persist.tile([P, P], BF16, tag="rt")
        mask_sb = persist.tile([P, P], BF16, tag="mask")
        ones_k_sb = persist.tile([P, 1], BF16, tag="ones_k")
        ones_1_sb = persist.tile([1, P], BF16, tag="ones_1")
        cos_sb = persist.tile([P, S], BF16, tag="cos")
        sin_sb = persist.tile([P, S], BF16, tag="sin")

        # resident weights ([wq | wk | wv] packed along columns)
        wqkv_sb = persist.tile([P, NT, (GH + 2) * HD], BF16,
                               tag="wqkv")                      # 24 KB/p

        # resident x (all 16 contraction tiles, full s)
        xall = persist.tile([P, NT, S], BF16, tag="x")          # 64 KB/p

        # roped projections; QR packs the 4 heads
        QR = persist.tile([P, GH, S], BF16, tag="qr")           # 16 KB/p
        KR = persist.tile([P, S], BF16, tag="kr")
        VT = persist.tile([P, S], BF16, tag="vt")  # V^T [d, s]
        VV = persist.tile([P, S], BF16, tag="vv")  # V [s, d] per kj tile
        # per-head attention outputs: separate tiles so phase 3's
        # dependency tracking is per (head, s-range), not whole-tensor
        OT = [persist.tile([P, S], BF16, tag=f"otq{h}", name=f"otq{h}")
              for h in range(GH)]                               # 16 KB/p

        # ---------------- Phase 1: projections + RoPE ----------------
        # DMA schedule: the sb=0 critical tiles (x[t], wqkv[t]) first
        # on the sync queue; consts + cos/sin next; x half-1 (needed
        # from sb=2) + V transposes + wo on the scalar queue.
        S1 = S // 2
        for t in range(NT):
            eng = nc.sync if t % 2 == 0 else nc.scalar
            eng.dma_start(out=xall[:, t, 0:S1],
                          in_=xT_d[t * P:(t + 1) * P, 0:S1])
            eng.dma_start(out=wqkv_sb[:, t, :],
                          in_=wqkv_d[t * P:(t + 1) * P, :])
        nc.sync.dma_start(out=rt_sb, in_=RT_d[:])
        nc.sync.dma_start(out=ones_k_sb, in_=ones_k_d[:])
        nc.sync.dma_start(out=ones_1_sb, in_=ones_1_d[:])
        nc.sync.dma_start(out=mask_sb, in_=mask01_d[:])
        nc.sync.dma_start(out=cos_sb, in_=cosT_d[:])
        nc.sync.dma_start(out=sin_sb, in_=sinT_d[:])
        for t in range(NT):
            nc.scalar.dma_start(out=xall[:, t, S1:S],
                                in_=xT_d[t * P:(t + 1) * P, S1:S])

        with (
            tc.tile_pool(name="p1w", bufs=2) as p1w,
            tc.tile_pool(name="p1ps", bufs=1, space="PSUM") as p1ps,
            tc.tile_pool(name="rotps", bufs=2, space="PSUM") as rotps,
        ):
            for sb in range(NB):
                ssl = slice(sb * SB, (sb + 1) * SB)
                ps = [p1ps.tile([P, SB], F32, tag=f"ps{i}",
                                name=f"ps{sb}_{i}")
                      for i in range(6)]
                for t in range(NT):
                    st_, sp_ = (t == 0), (t == NT - 1)
                    for o in range(6):   # q0..q3, k, v
                        nc.tensor.matmul(
                            ps[o], wqkv_sb[:, t, o * HD:(o + 1) * HD],
                            xall[:, t, ssl], start=st_, stop=sp_)
                for i in range(5):
                    raw = p1w.tile([P, SB], BF16, tag="raw",
                                   name=f"raw{sb}_{i}")
                    # alternate PSUM drains across ACT/DVE
                    if i % 2 == 0:
                        nc.scalar.copy(raw, ps[i])
                    else:
                        with nc.allow_low_precision(reason="bf16 qk"):
                            nc.vector.tensor_copy(raw, ps[i])
                    rot = rotps.tile([P, SB], F32, tag="rot",
                                     name=f"rot{sb}_{i}")
                    nc.tensor.matmul(rot, rt_sb, raw)
                    t1 = p1w.tile([P, SB], BF16, tag="t1",
                                  name=f"t1_{sb}_{i}")
                    nc.vector.tensor_mul(t1, raw, cos_sb[:, ssl])
                    t2 = p1w.tile([P, SB], BF16, tag="t2",
                                  name=f"t2_{sb}_{i}")
                    nc.vector.tensor_mul(t2, rot, sin_sb[:, ssl])
                    # final add on Pool: off the critical path (next
                    # reader is phase 2)
                    dst = QR[:, i, ssl] if i < GH else KR[:, ssl]
                    nc.gpsimd.tensor_add(dst, t1, t2)
                nc.scalar.copy(VT[:, ssl], ps[5])
                for tt in range(SB // P):
                    blk = sb * (SB // P) + tt
                    bs = slice(blk * P, (blk + 1) * P)
                    nc.scalar.dma_start(out=VV[:, bs], in_=VT[:, bs],
                                        transpose=True)

        # ---------------- Phase 2: attention ----------------
        # wo prefetch into wqkv_sb's slot (dead after phase 1); sync
        # queue is idle during attention.
        wo_sb = wqkv_sb.rearrange("p a b -> p (a b)")[:, 0:GH * S] \
            .rearrange("p (g e) -> p g e", g=GH)
        for hh in range(GH):
            nc.sync.dma_start(out=wo_sb[:, hh, :],
                              in_=wo_d[hh * P:(hh + 1) * P, :])

        with (
            tc.tile_pool(name="pp", bufs=3) as pp,
            tc.tile_pool(name="p2w", bufs=2) as p2w,
            tc.tile_pool(name="stps", bufs=2, space="PSUM") as stps,
            tc.tile_pool(name="otps", bufs=1, space="PSUM") as otps,
            tc.tile_pool(name="rsps", bufs=1, space="PSUM") as rsps,
        ):
            # pending_norm parts of the previous (qb, hf) block; each is
            # issued at a staggered point inside the next block so no
            # engine ever stalls on the chain.
            norm_pre = norm_rest = None
            for qb in range(NB):
                for hf in range(2):      # head pairs (0,1) and (2,3)
                    h0 = 2 * hf
                    nkj = 4 * (qb + 1)
                    ot_ps = otps.tile([P, 2, SB], F32, tag="ot",
                                      name=f"otp{qb}_{hf}")
                    rs_ps = rsps.tile([1, 2, SB], F32, tag="rs",
                                      name=f"rsp{qb}_{hf}")
                    if norm_pre is not None:
                        norm_pre()
                        norm_pre = None
                    pend = []  # [(kj, q0, p_sb)] awaiting PV+rowsum

                    def _pv(kj, q0, p_sb, qb=qb, nkj=nkj, ot_ps=ot_ps,
                            rs_ps=rs_ps):
                        first, last = (kj == 0), (kj == nkj - 1)
                        kb = slice(kj * P, (kj + 1) * P)
                        for hp in range(2):
                            nc.tensor.matmul(
                                ot_ps[:, hp, q0:], VV[:, kb],
                                p_sb[:, hp, q0:],
                                start=first, stop=last,
                                skip_group_check=True)
                        for hp in range(2):
                            nc.tensor.matmul(
                                rs_ps[:, hp, q0:], ones_k_sb,
                                p_sb[:, hp, q0:],
                                start=first, stop=last,
                                skip_group_check=True)

                    for kj in range(nkj):
                        j = kj - (nkj - 4)
                        q0 = 0 if j < 0 else P * j
                        kb = slice(kj * P, (kj + 1) * P)
                        qsl = slice(qb * SB + q0, (qb + 1) * SB)
                        st = stps.tile([P, 2, SB], F32, tag="st",
                                       name=f"st{qb}_{hf}_{kj}")
                        nc.tensor.matmul(st[:, 0, q0:], KR[:, kb],
                                         QR[:, h0, qsl])
                        nc.tensor.matmul(st[:, 1, q0:], KR[:, kb],
                                         QR[:, h0 + 1, qsl])
                        if kj == 3 and norm_rest is not None:
                            norm_rest()
                            norm_rest = None
                        # two-step software pipeline: PV of kj-2 lands
                        # here, so exp(kj) has ~2 PE iterations of slack
                        if len(pend) == 2:
                            _pv(*pend.pop(0))
                        p_sb = pp.tile([P, 2, SB], BF16, tag="p",
                                       name=f"p{qb}_{hf}_{kj}")
                        nc.scalar.activation(
                            p_sb[:, :, q0:], st[:, :, q0:],
                            mybir.ActivationFunctionType.Exp, scale=SCALE)
                        if j >= 0:
                            dsl = slice(q0, q0 + P)
                            with nc.allow_low_precision(
                                    reason="0/1 causal mask on bf16 P"):
                                nc.vector.tensor_mul(p_sb[:, 0, dsl],
                                                     p_sb[:, 0, dsl],
                                                     mask_sb)
                                nc.vector.tensor_mul(p_sb[:, 1, dsl],
                                                     p_sb[:, 1, dsl],
                                                     mask_sb)
                        pend.append((kj, q0, p_sb))
                    for pe_ in pend:
                        _pv(*pe_)

                    def _norm_pre(qb=qb, hf=hf, ot_ps=ot_ps, rs_ps=rs_ps):
                        # staging reads that free ot/rs for the next
                        # block -- DVE + ACT only, issued before the
                        # next block's first matmuls
                        u_sb = p2w.tile([P, 2, SB], BF16, tag="u",
                                        name=f"u{qb}_{hf}")
                        with nc.allow_low_precision(
                                reason="bf16 attention numerator"):
                            nc.vector.tensor_copy(u_sb, ot_ps)
                        rs_sb = p2w.tile([1, 2, SB], BF16, tag="rsb",
                                         name=f"rsb{qb}_{hf}")
                        with nc.allow_low_precision(
                                reason="bf16 rowsums"):
                            nc.vector.tensor_copy(rs_sb, rs_ps)
                        # spread the 1024 rowsums over 128 partitions
                        # (tiny SBUF->SBUF DMA) so the reciprocal is
                        # 128-lane instead of 1-lane, then gather back
                        rsT = p2w.tile([P, 2 * SB // P], BF16, tag="rst",
                                       name=f"rst{qb}_{hf}")
                        nc.sync.dma_start(
                            out=rsT, in_=rs_sb.rearrange("o a b -> o (a b)"))
                        recT = p2w.tile([P, 2 * SB // P], BF16, tag="rct",
                                        name=f"rct{qb}_{hf}")
                        with nc.allow_low_precision(
                                reason="bf16 1/rowsum; rel budget 2e-2"):
                            nc.vector.reciprocal(recT, rsT)
                        rec_sb = p2w.tile([1, 2, SB], BF16, tag="rcb",
                                          name=f"rcb{qb}_{hf}")
                        nc.sync.dma_start(
                            out=rec_sb.rearrange("o a b -> o (a b)"),
                            in_=recT)
                        _norm_pre.u_sb = u_sb
                        _norm_pre.rec_sb = rec_sb

                    def _norm_rest(qb=qb, hf=hf, h0=h0, pre=_norm_pre):
                        qsl = slice(qb * SB, (qb + 1) * SB)
                        # broadcast 1/rowsum via PE (borrows one st
                        # slot), then normalize on DVE
                        bc_ps = stps.tile([P, 2, SB], F32, tag="st",
                                          name=f"bc{qb}_{hf}")
                        for hp in range(2):
                            nc.tensor.matmul(bc_ps[:, hp, :], ones_1_sb,
                                             pre.rec_sb[:, hp, :])
                        with nc.allow_low_precision(
                                reason="bf16 normalized attention out"):
                            for hp in range(2):
                                nc.vector.tensor_mul(
                                    OT[h0 + hp][:, qsl],
                                    pre.u_sb[:, hp, :], bc_ps[:, hp, :])

                    norm_pre, norm_rest = _norm_pre, _norm_rest
            if norm_pre is not None:
                norm_pre()
            if norm_rest is not None:
                norm_rest()

            # ---------------- Phase 3: output projection ----------------
            # shares the stps pool (no PSUM pool-close barrier)
            for e in range(NT):
                for sp_ in range(2):          # s-block pairs
                    o_ps = stps.tile([P, 2, SB], F32, tag="st",
                                     name=f"o{e}_{sp_}")
                    for hh in range(GH):
                        for sbi in range(2):
                            sb = 2 * sp_ + sbi
                            ssl = slice(sb * SB, (sb + 1) * SB)
                            nc.tensor.matmul(
                                o_ps[:, sbi, :],
                                wo_sb[:, hh, e * P:(e + 1) * P],
                                OT[hh][:, ssl],
                                start=(hh == 0), stop=(hh == GH - 1))
                    oe = p2w.tile([P, 2, SB], BF16, tag="oe",
                                  name=f"oe{e}_{sp_}", bufs=3)
                    with nc.allow_low_precision(
                            reason="bf16 partial outputs; host sums "
                                   "in f32"):
                        if sp_ == 0:
                            nc.scalar.copy(oe, o_ps)
                        else:
                            nc.vector.tensor_copy(oe, o_ps)
                    nc.sync.dma_start(
                        out=out_d[e * P:(e + 1) * P,
                                  sp_ * 2 * SB:(sp_ + 1) * 2 * SB],
                        in_=oe.rearrange("p a b -> p (a b)"))

    _hoist_matmul_waits(nc)
    return nc


def _hoist_matmul_waits(nc):
    """Some engine instructions only support ONE sync-wait in the ISA
    encoding -- walrus puts all waits on one struct.  Hoist extra waits
    onto standalone single-wait EventSemaphores inserted right before
    the offending instruction on the same engine."""
    n_fixed = 0
    for fn in nc.m.functions:
        for blk in fn.blocks:
            out = []
            for inst in blk.instructions:
                si = inst.sync_info
                if (inst.opcode != "EventSemaphore" and si is not None
                        and si.on_wait is not None and len(si.on_wait) > 1):
                    waits = list(si.on_wait)
                    for wi, w in enumerate(waits[:-1]):
                        out.append(mybir.InstEventSemaphore(
                            name=f"hoistw_{inst.name}_{wi}", ins=[], outs=[],
                            sync_info=mybir.SyncInfo(on_wait=[w],
                                                     on_update=[]),
                            engine=inst.engine))
                    inst.sync_info = mybir.SyncInfo(
                        on_wait=[waits[-1]],
                        on_update=list(si.on_update or []))
                    n_fixed += 1
                out.append(inst)
            blk.instructions = out
    return n_fixed


def make_in_maps(x, cos, sin, wq, wk, wv, wo):
    cosT = np.ascontiguousarray(np.asarray(cos).T.astype(NPDT))
    sinT = np.ascontiguousarray(np.asarray(sin).T.astype(NPDT))
    xT = [np.ascontiguousarray(np.asarray(x[b]).T.astype(NPDT))
          for b in range(B)]
    wq, wk, wv, wo = (np.asarray(a).astype(NPDT) for a in (wq, wk, wv, wo))
    in_maps = []
    for c in range(NCORES):
        b, g = divmod(c, NKV)
        wqkv = np.concatenate([
            wq[:, g * GH * HD:(g + 1) * GH * HD],
            wk[:, g * HD:(g + 1) * HD],
            wv[:, g * HD:(g + 1) * HD]], axis=1)
        in_maps.append({
            "xT": xT[b],
            "wqkv": np.ascontiguousarray(wqkv),
            "wo": np.ascontiguousarray(wo[g * GH * HD:(g + 1) * GH * HD, :]),
            "cosT": cosT,
            "sinT": sinT,
        })
    return in_maps


_NC_CACHE = {}


def _get_nc():
    if "nc" not in _NC_CACHE:
        _NC_CACHE["nc"] = build_nc()
    return _NC_CACHE["nc"]


def run(x, cos, sin, wq, wk, wv, wo, **spmd_kwargs):
    nc = _get_nc()
    in_maps = make_in_maps(x, cos, sin, wq, wk, wv, wo)
    res = run_bass_kernel_spmd(nc, in_maps, core_ids=list(range(NCORES)),
                               **spmd_kwargs)
    outs = [np.asarray(res.results[c]["out"]) for c in range(NCORES)]
    full = np.empty((B, S, H), np.float32)
    for b in range(B):
        acc = outs[4 * b].astype(np.float32)
        for g in range(1, NKV):
            acc += outs[4 * b + g].astype(np.float32)
        full[b] = acc.T
    return full, res


def kernel(**inputs):
    out, _ = run(**inputs)
    return out


if __name__ == "__main__":
    import tempfile
    from concourse.bass_utils import compile_bir_kernel

    nc = build_nc()
    print("graph built OK")
    if os.environ.get("COMPILE_CHECK", "1") == "1":
        td = tempfile.mkdtemp(prefix="bass_compile_")
        neff = compile_bir_kernel(nc.to_json_bytes(), td, "kernel.neff")
        print(f"compiled OK: {neff}")


# revision 28
# speedup vs baseline: 1.0380x; 1.0380x over previous
"""Distributed Bass kernel for GQA causal attention (B=2, S=2048, H=2048,
NH=16, NKV=4, HD=128) on 8 TRN2 NeuronCores.

Sharding: core c (0..7) handles batch b = c//4 and kv-group g = c%4
(4 query heads + 1 kv head, GQA groups kept intact).  wq/wk/wv are
column-sharded, wo row-sharded; each core emits a partial output
[H, S] (transposed, bf16) and the host sums the 4 group-partials per
batch in f32.

v2 design notes (vs the f32r v1):
  - All matmuls in bf16: full-rate streaming at ANY moving width
    (f32r drops to 1/4 rate under 256 cols -- the causal-diagonal
    tiles), half DMA traffic / SBUF footprint / LDWEIGHTS size.
  - Matmul ISA limits respected: every matmul out is a 2D <=512-col
    single-PSUM-bank region; head-pairs share a [P, 2, SB] PSUM tile
    so ACT/DVE/Pool touch both heads with one (3D-AP) instruction.
  - Elementwise work spread across engines: ACT (exp, PSUM->SBUF
    staging), DVE (rot*sin, adds, 128-lane reciprocals, U staging),
    Pool==nc.gpsimd (raw*cos, causal mask multiply, final normalize).
  - V transposed to [s, d] 128-blocks by DMA-transpose (XBAR), not PE.
  - Attention runs kj-outer with a one-step software pipeline: the
    score matmul of tile kj+1 issues before the PV/rowsum of kj, so
    the ACT exp latency never stalls the PE.  Causal mask is a
    multiplicative 0/1 bf16 on P after exp (Pool engine).
  - Per-block softmax normalization (U copy, rowsum staging, PE
    broadcast matmuls, reciprocal, U*(1/r)) is deferred into the next
    block's kj loop -- fully off the PE critical path.
  - Output projection accumulates into [P, 2, SB] tiles (2 s-blocks),
    staged to SBUF alternating ACT/DVE, DMA'd out as bf16.
"""

import math
import os
import sys

import ml_dtypes
import numpy as np

sys.path.insert(0, "/opt/trn_rl_repo")

import concourse.bass as bass
import concourse.mybir as mybir
import concourse.tile as tile
from concourse.bass_utils import run_bass_kernel_spmd

B, S, H = 2, 2048, 2048
NH, NKV, HD = 16, 4, 128
NCORES = 8
GH = 4                # q-heads per core (one kv group)
P = 128
SB = 512              # s-block width (single PSUM bank of f32)
NB = S // SB          # 4 s-blocks
NT = S // P           # 16 partition tiles along s / h / e
SCALE = 1.0 / math.sqrt(HD)
F32 = mybir.dt.float32
BF16 = mybir.dt.bfloat16
NPDT = ml_dtypes.bfloat16


def _consts():
    # rotate_half as matmul: rot = RT.T @ q  (RT is the lhsT)
    RT = np.zeros((P, P), NPDT)
    idx = np.arange(64)
    RT[idx + 64, idx] = -1.0
    RT[idx, idx + 64] = 1.0
    # multiplicative causal mask for the diagonal P x P block:
    # P[kj, qi] valid iff kj <= qi
    kjl = np.arange(P)[:, None]
    qil = np.arange(P)[None, :]
    mask01 = (kjl <= qil).astype(NPDT)
    ones_k = np.ones((P, 1), NPDT)
    ones_1 = np.ones((1, P), NPDT)
    return RT, mask01, ones_k, ones_1


def build_nc():
    nc = bass.Bass()

    xT_d = nc.declare_dram_parameter("xT", [H, S], BF16, isOutput=False)
    # host packs [wq | wk | wv] -> one [H, 768] param (fewer DMA issues)
    wqkv_d = nc.declare_dram_parameter("wqkv", [H, (GH + 2) * HD], BF16,
                                       isOutput=False)
    wo_d = nc.declare_dram_parameter("wo", [GH * HD, H], BF16, isOutput=False)
    cosT_d = nc.declare_dram_parameter("cosT", [HD, S], BF16, isOutput=False)
    sinT_d = nc.declare_dram_parameter("sinT", [HD, S], BF16, isOutput=False)
    out_d = nc.declare_dram_parameter("out", [H, S], BF16, isOutput=True)

    RT_np, mask01_np, ones_k_np, ones_1_np = _consts()
    RT_d = nc.inline_tensor(RT_np, "rot_t")
    mask01_d = nc.inline_tensor(mask01_np, "mask01")
    ones_k_d = nc.inline_tensor(ones_k_np, "ones_k")
    ones_1_d = nc.inline_tensor(ones_1_np, "ones_1")

    with tile.TileContext(nc) as tc, \
         tc.tile_pool(name="persist", bufs=1) as persist:
        rt_sb = persist.tile([P, P], BF16, tag="rt")
        mask_sb = persist.tile([P, P], BF16, tag="mask")
        ones_k_sb = persist.tile([P, 1], BF16, tag="ones_k")
        ones_1_sb = persist.tile([1, P], BF16, tag="ones_1")
        cos_sb = persist.tile([P, S], BF16, tag="cos")
        sin_sb = persist.tile([P, S], BF16, tag="sin")

        # resident weights ([wq | wk | wv] packed along columns)
        wqkv_sb = persist.tile([P, NT, (GH + 2) * HD], BF16,
                               tag="wqkv")                      # 24 KB/p

        # resident x (all 16 contraction tiles, full s)
        xall = persist.tile([P, NT, S], BF16, tag="x")          # 64 KB/p

        # roped projections; QR packs the 4 heads
        QR = persist.tile([P, GH, S], BF16, tag="qr")           # 16 KB/p
        KR = persist.tile([P, S], BF16, tag="kr")
        VT = persist.tile([P, S], BF16, tag="vt")  # V^T [d, s]
        VV = persist.tile([P, S], BF16, tag="vv")  # V [s, d] per kj tile
        # per-head attention outputs: separate tiles so phase 3's
        # dependency tracking is per (head, s-range), not whole-tensor
        OT = [persist.tile([P, S], BF16, tag=f"otq{h}", name=f"otq{h}")
              for h in range(GH)]                               # 16 KB/p

        # ---------------- Phase 1: projections + RoPE ----------------
        # DMA schedule: the sb=0 critical tiles (x[t], wqkv[t]) first
        # on the sync queue; consts + cos/sin next; x half-1 (needed
        # from sb=2) + V transposes + wo on the scalar queue.
        S1 = S // 2
        for t in range(NT):
            eng = nc.sync if t % 2 == 0 else nc.scalar
            eng.dma_start(out=xall[:, t, 0:S1],
                          in_=xT_d[t * P:(t + 1) * P, 0:S1])
            eng.dma_start(out=wqkv_sb[:, t, :],
                          in_=wqkv_d[t * P:(t + 1) * P, :])
        nc.sync.dma_start(out=rt_sb, in_=RT_d[:])
        nc.sync.dma_start(out=ones_k_sb, in_=ones_k_d[:])
        nc.sync.dma_start(out=ones_1_sb, in_=ones_1_d[:])
        nc.sync.dma_start(out=mask_sb, in_=mask01_d[:])
        nc.sync.dma_start(out=cos_sb, in_=cosT_d[:])
        nc.sync.dma_start(out=sin_sb, in_=sinT_d[:])
        for t in range(NT):
            nc.scalar.dma_start(out=xall[:, t, S1:S],
                                in_=xT_d[t * P:(t + 1) * P, S1:S])

        with (
            tc.tile_pool(name="p1w", bufs=2) as p1w,
            tc.tile_pool(name="p1ps", bufs=1, space="PSUM") as p1ps,
            tc.tile_pool(name="rotps", bufs=2, space="PSUM") as rotps,
        ):
            for sb in range(NB):
                ssl = slice(sb * SB, (sb + 1) * SB)
                ps = [p1ps.tile([P, SB], F32, tag=f"ps{i}",
                                name=f"ps{sb}_{i}")
                      for i in range(6)]
                for t in range(NT):
                    st_, sp_ = (t == 0), (t == NT - 1)
                    for o in range(6):   # q0..q3, k, v
                        nc.tensor.matmul(
                            ps[o], wqkv_sb[:, t, o * HD:(o + 1) * HD],
                            xall[:, t, ssl], start=st_, stop=sp_)
                for i in range(5):
                    raw = p1w.tile([P, SB], BF16, tag="raw",
                                   name=f"raw{sb}_{i}")
                    # PSUM drains on DVE only: the scalar queue is a
                    # pure DMA-issue queue in phase 1, and a copy queued
                    # behind its DMA issues would delay the bank release
                    with nc.allow_low_precision(reason="bf16 qk"):
                        nc.vector.tensor_copy(raw, ps[i])
                    rot = rotps.tile([P, SB], F32, tag="rot",
                                     name=f"rot{sb}_{i}")
                    nc.tensor.matmul(rot, rt_sb, raw)
                    t1 = p1w.tile([P, SB], BF16, tag="t1",
                                  name=f"t1_{sb}_{i}")
                    nc.vector.tensor_mul(t1, raw, cos_sb[:, ssl])
                    t2 = p1w.tile([P, SB], BF16, tag="t2",
                                  name=f"t2_{sb}_{i}")
                    nc.vector.tensor_mul(t2, rot, sin_sb[:, ssl])
                    # final add on Pool: off the critical path (next
                    # reader is phase 2)
                    dst = QR[:, i, ssl] if i < GH else KR[:, ssl]
                    nc.gpsimd.tensor_add(dst, t1, t2)
                with nc.allow_low_precision(reason="bf16 v"):
                    nc.vector.tensor_copy(VT[:, ssl], ps[5])
                for tt in range(SB // P):
                    blk = sb * (SB // P) + tt
                    bs = slice(blk * P, (blk + 1) * P)
                    nc.scalar.dma_start(out=VV[:, bs], in_=VT[:, bs],
                                        transpose=True)

        # ---------------- Phase 2: attention ----------------
        # wo prefetch into wqkv_sb's slot (dead after phase 1); sync
        # queue is idle during attention.
        wo_sb = wqkv_sb.rearrange("p a b -> p (a b)")[:, 0:GH * S] \
            .rearrange("p (g e) -> p g e", g=GH)
        for hh in range(GH):
            nc.sync.dma_start(out=wo_sb[:, hh, :],
                              in_=wo_d[hh * P:(hh + 1) * P, :])

        with (
            tc.tile_pool(name="pp", bufs=3) as pp,
            tc.tile_pool(name="p2w", bufs=2) as p2w,
            tc.tile_pool(name="stps", bufs=2, space="PSUM") as stps,
            tc.tile_pool(name="otps", bufs=1, space="PSUM") as otps,
            tc.tile_pool(name="rsps", bufs=1, space="PSUM") as rsps,
        ):
            # pending_norm parts of the previous (qb, hf) block; each is
            # issued at a staggered point inside the next block so no
            # engine ever stalls on the chain.
            norm_pre = norm_rest = None
            for qb in range(NB):
                for hf in range(2):      # head pairs (0,1) and (2,3)
                    h0 = 2 * hf
                    nkj = 4 * (qb + 1)
                    ot_ps = otps.tile([P, 2, SB], F32, tag="ot",
                                      name=f"otp{qb}_{hf}")
                    rs_ps = rsps.tile([1, 2, SB], F32, tag="rs",
                                      name=f"rsp{qb}_{hf}")
                    if norm_pre is not None:
                        norm_pre()
                        norm_pre = None
                    pend = []  # [(kj, q0, p_sb)] awaiting PV+rowsum

                    def _pv(kj, q0, p_sb, qb=qb, nkj=nkj, ot_ps=ot_ps,
                            rs_ps=rs_ps):
                        first, last = (kj == 0), (kj == nkj - 1)
                        kb = slice(kj * P, (kj + 1) * P)
                        for hp in range(2):
                            nc.tensor.matmul(
                                ot_ps[:, hp, q0:], VV[:, kb],
                                p_sb[:, hp, q0:],
                                start=first, stop=last,
                                skip_group_check=True)
                        for hp in range(2):
                            nc.tensor.matmul(
                                rs_ps[:, hp, q0:], ones_k_sb,
                                p_sb[:, hp, q0:],
                                start=first, stop=last,
                                skip_group_check=True)

                    for kj in range(nkj):
                        j = kj - (nkj - 4)
                        q0 = 0 if j < 0 else P * j
                        kb = slice(kj * P, (kj + 1) * P)
                        qsl = slice(qb * SB + q0, (qb + 1) * SB)
                        st = stps.tile([P, 2, SB], F32, tag="st",
                                       name=f"st{qb}_{hf}_{kj}")
                        nc.tensor.matmul(st[:, 0, q0:], KR[:, kb],
                                         QR[:, h0, qsl])
                        nc.tensor.matmul(st[:, 1, q0:], KR[:, kb],
                                         QR[:, h0 + 1, qsl])
                        if kj == 3 and norm_rest is not None:
                            norm_rest()
                            norm_rest = None
                        # two-step software pipeline: PV of kj-2 lands
                        # here, so exp(kj) has ~2 PE iterations of slack
                        if len(pend) == 2:
                            _pv(*pend.pop(0))
                        p_sb = pp.tile([P, 2, SB], BF16, tag="p",
                                       name=f"p{qb}_{hf}_{kj}")
                        nc.scalar.activation(
                            p_sb[:, :, q0:], st[:, :, q0:],
                            mybir.ActivationFunctionType.Exp, scale=SCALE)
                        if j >= 0:
                            dsl = slice(q0, q0 + P)
                            with nc.allow_low_precision(
                                    reason="0/1 causal mask on bf16 P"):
                                nc.vector.tensor_mul(p_sb[:, 0, dsl],
                                                     p_sb[:, 0, dsl],
                                                     mask_sb)
                                nc.vector.tensor_mul(p_sb[:, 1, dsl],
                                                     p_sb[:, 1, dsl],
                                                     mask_sb)
                        pend.append((kj, q0, p_sb))
                    for pe_ in pend:
                        _pv(*pe_)

                    def _norm_pre(qb=qb, hf=hf, ot_ps=ot_ps, rs_ps=rs_ps):
                        # staging reads that free ot/rs for the next
                        # block -- DVE + ACT only, issued before the
                        # next block's first matmuls
                        u_sb = p2w.tile([P, 2, SB], BF16, tag="u",
                                        name=f"u{qb}_{hf}")
                        with nc.allow_low_precision(
                                reason="bf16 attention numerator"):
                            nc.vector.tensor_copy(u_sb, ot_ps)
                        rs_sb = p2w.tile([1, 2, SB], BF16, tag="rsb",
                                         name=f"rsb{qb}_{hf}")
                        with nc.allow_low_precision(
                                reason="bf16 rowsums"):
                            nc.vector.tensor_copy(rs_sb, rs_ps)
                        # spread the 1024 rowsums over 128 partitions
                        # (tiny SBUF->SBUF DMA) so the reciprocal is
                        # 128-lane instead of 1-lane, then gather back
                        rsT = p2w.tile([P, 2 * SB // P], BF16, tag="rst",
                                       name=f"rst{qb}_{hf}")
                        nc.sync.dma_start(
                            out=rsT, in_=rs_sb.rearrange("o a b -> o (a b)"))
                        recT = p2w.tile([P, 2 * SB // P], BF16, tag="rct",
                                        name=f"rct{qb}_{hf}")
                        with nc.allow_low_precision(
                                reason="bf16 1/rowsum; rel budget 2e-2"):
                            nc.vector.reciprocal(recT, rsT)
                        rec_sb = p2w.tile([1, 2, SB], BF16, tag="rcb",
                                          name=f"rcb{qb}_{hf}")
                        nc.sync.dma_start(
                            out=rec_sb.rearrange("o a b -> o (a b)"),
                            in_=recT)
                        _norm_pre.u_sb = u_sb
                        _norm_pre.rec_sb = rec_sb

                    def _norm_rest(qb=qb, hf=hf, h0=h0, pre=_norm_pre):
                        qsl = slice(qb * SB, (qb + 1) * SB)
                        # broadcast 1/rowsum via PE (borrows one st
                        # slot), then normalize on DVE
                        bc_ps = stps.tile([P, 2, SB], F32, tag="st",
                                          name=f"bc{qb}_{hf}")
                        for hp in range(2):
                            nc.tensor.matmul(bc_ps[:, hp, :], ones_1_sb,
                                             pre.rec_sb[:, hp, :])
                        with nc.allow_low_precision(
                                reason="bf16 normalized attention out"):
                            for hp in range(2):
                                nc.vector.tensor_mul(
                                    OT[h0 + hp][:, qsl],
                                    pre.u_sb[:, hp, :], bc_ps[:, hp, :])

                    norm_pre, norm_rest = _norm_pre, _norm_rest
            if norm_pre is not None:
                norm_pre()
            if norm_rest is not None:
                norm_rest()

            # ---------------- Phase 3: output projection ----------------
            # shares the stps pool (no PSUM pool-close barrier)
            for e in range(NT):
                for sp_ in range(2):          # s-block pairs
                    o_ps = stps.tile([P, 2, SB], F32, tag="st",
                                     name=f"o{e}_{sp_}")
                    for hh in range(GH):
                        for sbi in range(2):
                            sb = 2 * sp_ + sbi
                            ssl = slice(sb * SB, (sb + 1) * SB)
                            nc.tensor.matmul(
                                o_ps[:, sbi, :],
                                wo_sb[:, hh, e * P:(e + 1) * P],
                                OT[hh][:, ssl],
                                start=(hh == 0), stop=(hh == GH - 1))
                    oe = p2w.tile([P, 2, SB], BF16, tag="oe",
                                  name=f"oe{e}_{sp_}", bufs=3)
                    with nc.allow_low_precision(
                            reason="bf16 partial outputs; host sums "
                                   "in f32"):
                        if sp_ == 0:
                            nc.scalar.copy(oe, o_ps)
                        else:
                            nc.vector.tensor_copy(oe, o_ps)
                    nc.sync.dma_start(
                        out=out_d[e * P:(e + 1) * P,
                                  sp_ * 2 * SB:(sp_ + 1) * 2 * SB],
                        in_=oe.rearrange("p a b -> p (a b)"))

    _hoist_matmul_waits(nc)
    return nc


def _hoist_matmul_waits(nc):
    """Some engine instructions only support ONE sync-wait in the ISA
    encoding -- walrus puts all waits on one struct.  Hoist extra waits
    onto standalone single-wait EventSemaphores inserted right before
    the offending instruction on the same engine."""
    n_fixed = 0
    for fn in nc.m.functions:
        for blk in fn.blocks:
            out = []
            for inst in blk.instructions:
                si = inst.sync_info
                if (inst.opcode != "EventSemaphore" and si is not None
                        and si.on_wait is not None and len(si.on_wait) > 1):
                    waits = list(si.on_wait)
                    for wi, w in enumerate(waits[:-1]):
                        out.append(mybir.InstEventSemaphore(
                            name=f"hoistw_{inst.name}_{wi}", ins=[], outs=[],
                            sync_info=mybir.SyncInfo(on_wait=[w],
                                                     on_update=[]),
                            engine=inst.engine))
                    inst.sync_info = mybir.SyncInfo(
                        on_wait=[waits[-1]],
                        on_update=list(si.on_update or []))
                    n_fixed += 1
                out.append(inst)
            blk.instructions = out
    return n_fixed


def make_in_maps(x, cos, sin, wq, wk, wv, wo):
    cosT = np.ascontiguousarray(np.asarray(cos).T.astype(NPDT))
    sinT = np.ascontiguousarray(np.asarray(sin).T.astype(NPDT))
    xT = [np.ascontiguousarray(np.asarray(x[b]).T.astype(NPDT))
          for b in range(B)]
    wq, wk, wv, wo = (np.asarray(a).astype(NPDT) for a in (wq, wk, wv, wo))
    in_maps = []
    for c in range(NCORES):
        b, g = divmod(c, NKV)
        wqkv = np.concatenate([
            wq[:, g * GH * HD:(g + 1) * GH * HD],
            wk[:, g * HD:(g + 1) * HD],
            wv[:, g * HD:(g + 1) * HD]], axis=1)
        in_maps.append({
            "xT": xT[b],
            "wqkv": np.ascontiguousarray(wqkv),
            "wo": np.ascontiguousarray(wo[g * GH * HD:(g + 1) * GH * HD, :]),
            "cosT": cosT,
            "sinT": sinT,
        })
    return in_maps


_NC_CACHE = {}


def _get_nc():
    if "nc" not in _NC_CACHE:
        _NC_CACHE["nc"] = build_nc()
    return _NC_CACHE["nc"]


def run(x, cos, sin, wq, wk, wv, wo, **spmd_kwargs):
    nc = _get_nc()
    in_maps = make_in_maps(x, cos, sin, wq, wk, wv, wo)
    res = run_bass_kernel_spmd(nc, in_maps, core_ids=list(range(NCORES)),
                               **spmd_kwargs)
    outs = [np.asarray(res.results[c]["out"]) for c in range(NCORES)]
    full = np.empty((B, S, H), np.float32)
    for b in range(B):
        acc = outs[4 * b].astype(np.float32)
        for g in range(1, NKV):
            acc += outs[4 * b + g].astype(np.float32)
        full[b] = acc.T
    return full, res


def kernel(**inputs):
    out, _ = run(**inputs)
    return out


if __name__ == "__main__":
    import tempfile
    from concourse.bass_utils import compile_bir_kernel

    nc = build_nc()
    print("graph built OK")
    if os.environ.get("COMPILE_CHECK", "1") == "1":
        td = tempfile.mkdtemp(prefix="bass_compile_")
        neff = compile_bir_kernel(nc.to_json_bytes(), td, "kernel.neff")
        print(f"compiled OK: {neff}")


# revision 32
# speedup vs baseline: 1.0529x; 1.0144x over previous
"""Distributed Bass kernel for GQA causal attention (B=2, S=2048, H=2048,
NH=16, NKV=4, HD=128) on 8 TRN2 NeuronCores.

Sharding: core c (0..7) handles batch b = c//4 and kv-group g = c%4
(4 query heads + 1 kv head, GQA groups kept intact).  wq/wk/wv are
column-sharded, wo row-sharded; each core emits a partial output
[H, S] (transposed, bf16) and the host sums the 4 group-partials per
batch in f32.

v2 design notes (vs the f32r v1):
  - All matmuls in bf16: full-rate streaming at ANY moving width
    (f32r drops to 1/4 rate under 256 cols -- the causal-diagonal
    tiles), half DMA traffic / SBUF footprint / LDWEIGHTS size.
  - Matmul ISA limits respected: every matmul out is a 2D <=512-col
    single-PSUM-bank region; head-pairs share a [P, 2, SB] PSUM tile
    so ACT/DVE/Pool touch both heads with one (3D-AP) instruction.
  - Elementwise work spread across engines: ACT (exp, PSUM->SBUF
    staging), DVE (rot*sin, adds, 128-lane reciprocals, U staging),
    Pool==nc.gpsimd (raw*cos, causal mask multiply, final normalize).
  - V transposed to [s, d] 128-blocks by DMA-transpose (XBAR), not PE.
  - Attention runs kj-outer with a one-step software pipeline: the
    score matmul of tile kj+1 issues before the PV/rowsum of kj, so
    the ACT exp latency never stalls the PE.  Causal mask is a
    multiplicative 0/1 bf16 on P after exp (Pool engine).
  - Per-block softmax normalization (U copy, rowsum staging, PE
    broadcast matmuls, reciprocal, U*(1/r)) is deferred into the next
    block's kj loop -- fully off the PE critical path.
  - Output projection accumulates into [P, 2, SB] tiles (2 s-blocks),
    staged to SBUF alternating ACT/DVE, DMA'd out as bf16.
"""

import math
import os
import sys

import ml_dtypes
import numpy as np

sys.path.insert(0, "/opt/trn_rl_repo")

import concourse.bass as bass
import concourse.mybir as mybir
import concourse.tile as tile
from concourse.bass_utils import run_bass_kernel_spmd

B, S, H = 2, 2048, 2048
NH, NKV, HD = 16, 4, 128
NCORES = 8
GH = 4                # q-heads per core (one kv group)
P = 128
SB = 512              # s-block width (single PSUM bank of f32)
NB = S // SB          # 4 s-blocks
NT = S // P           # 16 partition tiles along s / h / e
SCALE = 1.0 / math.sqrt(HD)
F32 = mybir.dt.float32
BF16 = mybir.dt.bfloat16
NPDT = ml_dtypes.bfloat16


def _consts():
    # rotate_half as matmul: rot = RT.T @ q  (RT is the lhsT)
    RT = np.zeros((P, P), NPDT)
    idx = np.arange(64)
    RT[idx + 64, idx] = -1.0
    RT[idx, idx + 64] = 1.0
    # multiplicative causal mask for the diagonal P x P block:
    # P[kj, qi] valid iff kj <= qi
    kjl = np.arange(P)[:, None]
    qil = np.arange(P)[None, :]
    mask01 = (kjl <= qil).astype(NPDT)
    ones_k = np.ones((P, 1), NPDT)
    ones_1 = np.ones((1, P), NPDT)
    return RT, mask01, ones_k, ones_1


def build_nc():
    nc = bass.Bass()

    xT_d = nc.declare_dram_parameter("xT", [H, S], BF16, isOutput=False)
    # host packs [wq | wk | wv] -> one [H, 768] param (fewer DMA issues)
    wqkv_d = nc.declare_dram_parameter("wqkv", [H, (GH + 2) * HD], BF16,
                                       isOutput=False)
    wo_d = nc.declare_dram_parameter("wo", [GH * HD, H], BF16, isOutput=False)
    cosT_d = nc.declare_dram_parameter("cosT", [HD, S], BF16, isOutput=False)
    sinT_d = nc.declare_dram_parameter("sinT", [HD, S], BF16, isOutput=False)
    out_d = nc.declare_dram_parameter("out", [H, S], BF16, isOutput=True)

    RT_np, mask01_np, ones_k_np, ones_1_np = _consts()
    RT_d = nc.inline_tensor(RT_np, "rot_t")
    mask01_d = nc.inline_tensor(mask01_np, "mask01")
    ones_k_d = nc.inline_tensor(ones_k_np, "ones_k")
    ones_1_d = nc.inline_tensor(ones_1_np, "ones_1")

    with tile.TileContext(nc) as tc, \
         tc.tile_pool(name="persist", bufs=1) as persist:
        rt_sb = persist.tile([P, P], BF16, tag="rt")
        mask_sb = persist.tile([P, P], BF16, tag="mask")
        ones_k_sb = persist.tile([P, 1], BF16, tag="ones_k")
        ones_1_sb = persist.tile([1, P], BF16, tag="ones_1")
        cos_sb = persist.tile([P, S], BF16, tag="cos")
        sin_sb = persist.tile([P, S], BF16, tag="sin")

        # resident weights ([wq | wk | wv] packed along columns)
        wqkv_sb = persist.tile([P, NT, (GH + 2) * HD], BF16,
                               tag="wqkv")                      # 24 KB/p

        # resident x (all 16 contraction tiles, full s)
        xall = persist.tile([P, NT, S], BF16, tag="x")          # 64 KB/p

        # roped projections; QR packs the 4 heads
        QR = persist.tile([P, GH, S], BF16, tag="qr")           # 16 KB/p
        KR = persist.tile([P, S], BF16, tag="kr")
        VT = persist.tile([P, S], BF16, tag="vt")  # V^T [d, s]
        VV = persist.tile([P, S], BF16, tag="vv")  # V [s, d] per kj tile
        # per-head attention outputs: separate tiles so phase 3's
        # dependency tracking is per (head, s-range), not whole-tensor
        OT = [persist.tile([P, S], BF16, tag=f"otq{h}", name=f"otq{h}")
              for h in range(GH)]                               # 16 KB/p

        # ---------------- Phase 1: projections + RoPE ----------------
        # DMA schedule: the sb=0 critical tiles (x[t], wqkv[t]) first
        # on the sync queue; consts + cos/sin next; x half-1 (needed
        # from sb=2) + V transposes + wo on the scalar queue.
        S1 = S // 2
        for t in range(NT):
            eng = nc.sync if t % 2 == 0 else nc.scalar
            eng.dma_start(out=xall[:, t, 0:S1],
                          in_=xT_d[t * P:(t + 1) * P, 0:S1])
            eng.dma_start(out=wqkv_sb[:, t, :],
                          in_=wqkv_d[t * P:(t + 1) * P, :])
            if t % 2 == 1:
                # slot x half-1 issues between the critical half-0
                # pairs so their transfers finish before sb=2
                tt = t // 2
                nc.scalar.dma_start(out=xall[:, tt, S1:S],
                                    in_=xT_d[tt * P:(tt + 1) * P, S1:S])
        nc.sync.dma_start(out=rt_sb, in_=RT_d[:])
        nc.sync.dma_start(out=ones_k_sb, in_=ones_k_d[:])
        nc.sync.dma_start(out=ones_1_sb, in_=ones_1_d[:])
        nc.sync.dma_start(out=mask_sb, in_=mask01_d[:])
        nc.sync.dma_start(out=cos_sb, in_=cosT_d[:])
        nc.sync.dma_start(out=sin_sb, in_=sinT_d[:])
        for t in range(NT // 2, NT):
            nc.scalar.dma_start(out=xall[:, t, S1:S],
                                in_=xT_d[t * P:(t + 1) * P, S1:S])

        with (
            tc.tile_pool(name="p1w", bufs=2) as p1w,
            tc.tile_pool(name="p1ps", bufs=1, space="PSUM") as p1ps,
            tc.tile_pool(name="rotps", bufs=2, space="PSUM") as rotps,
        ):
            for sb in range(NB):
                ssl = slice(sb * SB, (sb + 1) * SB)
                ps = [p1ps.tile([P, SB], F32, tag=f"ps{i}",
                                name=f"ps{sb}_{i}")
                      for i in range(6)]
                for t in range(NT):
                    st_, sp_ = (t == 0), (t == NT - 1)
                    for o in range(6):   # q0..q3, k, v
                        nc.tensor.matmul(
                            ps[o], wqkv_sb[:, t, o * HD:(o + 1) * HD],
                            xall[:, t, ssl], start=st_, stop=sp_)
                for i in range(5):
                    raw = p1w.tile([P, SB], BF16, tag="raw",
                                   name=f"raw{sb}_{i}")
                    # PSUM drains on DVE only: the scalar queue is a
                    # pure DMA-issue queue in phase 1, and a copy queued
                    # behind its DMA issues would delay the bank release
                    with nc.allow_low_precision(reason="bf16 qk"):
                        nc.vector.tensor_copy(raw, ps[i])
                    rot = rotps.tile([P, SB], F32, tag="rot",
                                     name=f"rot{sb}_{i}")
                    nc.tensor.matmul(rot, rt_sb, raw)
                    t1 = p1w.tile([P, SB], BF16, tag="t1",
                                  name=f"t1_{sb}_{i}")
                    nc.vector.tensor_mul(t1, raw, cos_sb[:, ssl])
                    t2 = p1w.tile([P, SB], BF16, tag="t2",
                                  name=f"t2_{sb}_{i}")
                    nc.vector.tensor_mul(t2, rot, sin_sb[:, ssl])
                    # final add on Pool: off the critical path (next
                    # reader is phase 2)
                    dst = QR[:, i, ssl] if i < GH else KR[:, ssl]
                    nc.gpsimd.tensor_add(dst, t1, t2)
                with nc.allow_low_precision(reason="bf16 v"):
                    nc.vector.tensor_copy(VT[:, ssl], ps[5])
                for tt in range(SB // P):
                    blk = sb * (SB // P) + tt
                    bs = slice(blk * P, (blk + 1) * P)
                    nc.scalar.dma_start(out=VV[:, bs], in_=VT[:, bs],
                                        transpose=True)

        # ---------------- Phase 2: attention ----------------
        # wo prefetch into wqkv_sb's slot (dead after phase 1); sync
        # queue is idle during attention.
        wo_sb = wqkv_sb.rearrange("p a b -> p (a b)")[:, 0:GH * S] \
            .rearrange("p (g e) -> p g e", g=GH)
        for hh in range(GH):
            nc.sync.dma_start(out=wo_sb[:, hh, :],
                              in_=wo_d[hh * P:(hh + 1) * P, :])

        with (
            tc.tile_pool(name="pp", bufs=3) as pp,
            tc.tile_pool(name="p2w", bufs=2) as p2w,
            tc.tile_pool(name="stps", bufs=2, space="PSUM") as stps,
            tc.tile_pool(name="otps", bufs=1, space="PSUM") as otps,
            tc.tile_pool(name="rsps", bufs=1, space="PSUM") as rsps,
        ):
            # pending_norm parts of the previous (qb, hf) block; each is
            # issued at a staggered point inside the next block so no
            # engine ever stalls on the chain.
            norm_pre = norm_rest = None
            for qb in range(NB):
                for hf in range(2):      # head pairs (0,1) and (2,3)
                    h0 = 2 * hf
                    nkj = 4 * (qb + 1)
                    ot_ps = otps.tile([P, 2, SB], F32, tag="ot",
                                      name=f"otp{qb}_{hf}")
                    rs_ps = rsps.tile([1, 2, SB], F32, tag="rs",
                                      name=f"rsp{qb}_{hf}")
                    if norm_pre is not None:
                        norm_pre()
                        norm_pre = None
                    pend = []  # [(kj, q0, p_sb)] awaiting PV+rowsum

                    def _pv(kj, q0, p_sb, qb=qb, nkj=nkj, ot_ps=ot_ps,
                            rs_ps=rs_ps):
                        first, last = (kj == 0), (kj == nkj - 1)
                        kb = slice(kj * P, (kj + 1) * P)
                        for hp in range(2):
                            nc.tensor.matmul(
                                ot_ps[:, hp, q0:], VV[:, kb],
                                p_sb[:, hp, q0:],
                                start=first, stop=last,
                                skip_group_check=True)
                        for hp in range(2):
                            nc.tensor.matmul(
                                rs_ps[:, hp, q0:], ones_k_sb,
                                p_sb[:, hp, q0:],
                                start=first, stop=last,
                                skip_group_check=True)

                    for kj in range(nkj):
                        j = kj - (nkj - 4)
                        q0 = 0 if j < 0 else P * j
                        kb = slice(kj * P, (kj + 1) * P)
                        qsl = slice(qb * SB + q0, (qb + 1) * SB)
                        st = stps.tile([P, 2, SB], F32, tag="st",
                                       name=f"st{qb}_{hf}_{kj}")
                        nc.tensor.matmul(st[:, 0, q0:], KR[:, kb],
                                         QR[:, h0, qsl])
                        nc.tensor.matmul(st[:, 1, q0:], KR[:, kb],
                                         QR[:, h0 + 1, qsl])
                        if kj == 3 and norm_rest is not None:
                            norm_rest()
                            norm_rest = None
                        # two-step software pipeline: PV of kj-2 lands
                        # here, so exp(kj) has ~2 PE iterations of slack
                        if len(pend) == 2:
                            _pv(*pend.pop(0))
                        p_sb = pp.tile([P, 2, SB], BF16, tag="p",
                                       name=f"p{qb}_{hf}_{kj}")
                        nc.scalar.activation(
                            p_sb[:, :, q0:], st[:, :, q0:],
                            mybir.ActivationFunctionType.Exp, scale=SCALE)
                        if j >= 0:
                            # masks on Pool: DVE must stay free for the
                            # boundary ot/rs drains
                            dsl = slice(q0, q0 + P)
                            with nc.allow_low_precision(
                                    reason="0/1 causal mask on bf16 P"):
                                nc.gpsimd.tensor_mul(p_sb[:, 0, dsl],
                                                     p_sb[:, 0, dsl],
                                                     mask_sb)
                                nc.gpsimd.tensor_mul(p_sb[:, 1, dsl],
                                                     p_sb[:, 1, dsl],
                                                     mask_sb)
                        pend.append((kj, q0, p_sb))
                    for pe_ in pend:
                        _pv(*pe_)

                    def _norm_pre(qb=qb, hf=hf, ot_ps=ot_ps, rs_ps=rs_ps):
                        # staging reads that free ot/rs for the next
                        # block -- DVE + ACT only, issued before the
                        # next block's first matmuls
                        u_sb = p2w.tile([P, 2, SB], BF16, tag="u",
                                        name=f"u{qb}_{hf}")
                        with nc.allow_low_precision(
                                reason="bf16 attention numerator"):
                            nc.vector.tensor_copy(u_sb, ot_ps)
                        rs_sb = p2w.tile([1, 2, SB], BF16, tag="rsb",
                                         name=f"rsb{qb}_{hf}")
                        with nc.allow_low_precision(
                                reason="bf16 rowsums"):
                            nc.vector.tensor_copy(rs_sb, rs_ps)
                        # spread the 1024 rowsums over 128 partitions
                        # (tiny SBUF->SBUF DMA) so the reciprocal is
                        # 128-lane instead of 1-lane, then gather back
                        rsT = p2w.tile([P, 2 * SB // P], BF16, tag="rst",
                                       name=f"rst{qb}_{hf}")
                        nc.sync.dma_start(
                            out=rsT, in_=rs_sb.rearrange("o a b -> o (a b)"))
                        recT = p2w.tile([P, 2 * SB // P], BF16, tag="rct",
                                        name=f"rct{qb}_{hf}")
                        with nc.allow_low_precision(
                                reason="bf16 1/rowsum; rel budget 2e-2"):
                            nc.vector.reciprocal(recT, rsT)
                        rec_sb = p2w.tile([1, 2, SB], BF16, tag="rcb",
                                          name=f"rcb{qb}_{hf}")
                        nc.sync.dma_start(
                            out=rec_sb.rearrange("o a b -> o (a b)"),
                            in_=recT)
                        _norm_pre.u_sb = u_sb
                        _norm_pre.rec_sb = rec_sb

                    def _norm_rest(qb=qb, hf=hf, h0=h0, pre=_norm_pre):
                        qsl = slice(qb * SB, (qb + 1) * SB)
                        # broadcast 1/rowsum via PE (borrows one st
                        # slot), then normalize on DVE
                        bc_ps = stps.tile([P, 2, SB], F32, tag="st",
                                          name=f"bc{qb}_{hf}")
                        for hp in range(2):
                            nc.tensor.matmul(bc_ps[:, hp, :], ones_1_sb,
                                             pre.rec_sb[:, hp, :])
                        with nc.allow_low_precision(
                                reason="bf16 normalized attention out"):
                            for hp in range(2):
                                nc.vector.tensor_mul(
                                    OT[h0 + hp][:, qsl],
                                    pre.u_sb[:, hp, :], bc_ps[:, hp, :])

                    norm_pre, norm_rest = _norm_pre, _norm_rest
            if norm_pre is not None:
                norm_pre()
                norm_pre = None

            # ---------------- Phase 3: output projection ----------------
            # Shares the stps pool (no PSUM pool-close barrier).  The
            # s-range [0:1024] pass runs FIRST: it doesn't depend on the
            # last attention block (s 1536:2048), whose norm_rest (PE
            # broadcast matmuls waiting on the reciprocal round-trip) is
            # issued between the passes so the PE never idles on it.
            for sp_ in range(2):
                if sp_ == 1 and norm_rest is not None:
                    norm_rest()
                    norm_rest = None
                for e in range(NT):
                    o_ps = stps.tile([P, 2, SB], F32, tag="st",
                                     name=f"o{e}_{sp_}")
                    for hh in range(GH):
                        for sbi in range(2):
                            sb = 2 * sp_ + sbi
                            ssl = slice(sb * SB, (sb + 1) * SB)
                            nc.tensor.matmul(
                                o_ps[:, sbi, :],
                                wo_sb[:, hh, e * P:(e + 1) * P],
                                OT[hh][:, ssl],
                                start=(hh == 0), stop=(hh == GH - 1))
                    oe = p2w.tile([P, 2, SB], BF16, tag="oe",
                                  name=f"oe{e}_{sp_}", bufs=3)
                    with nc.allow_low_precision(
                            reason="bf16 partial outputs; host sums "
                                   "in f32"):
                        if e % 2 == 0:
                            nc.scalar.copy(oe, o_ps)
                        else:
                            nc.vector.tensor_copy(oe, o_ps)
                    nc.sync.dma_start(
                        out=out_d[e * P:(e + 1) * P,
                                  sp_ * 2 * SB:(sp_ + 1) * 2 * SB],
                        in_=oe.rearrange("p a b -> p (a b)"))

    _hoist_matmul_waits(nc)
    return nc


def _hoist_matmul_waits(nc):
    """Some engine instructions only support ONE sync-wait in the ISA
    encoding -- walrus puts all waits on one struct.  Hoist extra waits
    onto standalone single-wait EventSemaphores inserted right before
    the offending instruction on the same engine."""
    n_fixed = 0
    for fn in nc.m.functions:
        for blk in fn.blocks:
            out = []
            for inst in blk.instructions:
                si = inst.sync_info
                if (inst.opcode != "EventSemaphore" and si is not None
                        and si.on_wait is not None and len(si.on_wait) > 1):
                    waits = list(si.on_wait)
                    for wi, w in enumerate(waits[:-1]):
                        out.append(mybir.InstEventSemaphore(
                            name=f"hoistw_{inst.name}_{wi}", ins=[], outs=[],
                            sync_info=mybir.SyncInfo(on_wait=[w],
                                                     on_update=[]),
                            engine=inst.engine))
                    inst.sync_info = mybir.SyncInfo(
                        on_wait=[waits[-1]],
                        on_update=list(si.on_update or []))
                    n_fixed += 1
                out.append(inst)
            blk.instructions = out
    return n_fixed


def make_in_maps(x, cos, sin, wq, wk, wv, wo):
    cosT = np.ascontiguousarray(np.asarray(cos).T.astype(NPDT))
    sinT = np.ascontiguousarray(np.asarray(sin).T.astype(NPDT))
    xT = [np.ascontiguousarray(np.asarray(x[b]).T.astype(NPDT))
          for b in range(B)]
    wq, wk, wv, wo = (np.asarray(a).astype(NPDT) for a in (wq, wk, wv, wo))
    in_maps = []
    for c in range(NCORES):
        b, g = divmod(c, NKV)
        wqkv = np.concatenate([
            wq[:, g * GH * HD:(g + 1) * GH * HD],
            wk[:, g * HD:(g + 1) * HD],
            wv[:, g * HD:(g + 1) * HD]], axis=1)
        in_maps.append({
            "xT": xT[b],
            "wqkv": np.ascontiguousarray(wqkv),
            "wo": np.ascontiguousarray(wo[g * GH * HD:(g + 1) * GH * HD, :]),
            "cosT": cosT,
            "sinT": sinT,
        })
    return in_maps


_NC_CACHE = {}


def _get_nc():
    if "nc" not in _NC_CACHE:
        _NC_CACHE["nc"] = build_nc()
    return _NC_CACHE["nc"]


def run(x, cos, sin, wq, wk, wv, wo, **spmd_kwargs):
    nc = _get_nc()
    in_maps = make_in_maps(x, cos, sin, wq, wk, wv, wo)
    res = run_bass_kernel_spmd(nc, in_maps, core_ids=list(range(NCORES)),
                               **spmd_kwargs)
    outs = [np.asarray(res.results[c]["out"]) for c in range(NCORES)]
    full = np.empty((B, S, H), np.float32)
    for b in range(B):
        acc = outs[4 * b].astype(np.float32)
        for g in range(1, NKV):
            acc += outs[4 * b + g].astype(np.float32)
        full[b] = acc.T
    return full, res


def kernel(**inputs):
    out, _ = run(**inputs)
    return out


if __name__ == "__main__":
    import tempfile
    from concourse.bass_utils import compile_bir_kernel

    nc = build_nc()
    print("graph built OK")
    if os.environ.get("COMPILE_CHECK", "1") == "1":
        td = tempfile.mkdtemp(prefix="bass_compile_")
        neff = compile_bir_kernel(nc.to_json_bytes(), td, "kernel.neff")
        print(f"compiled OK: {neff}")
